# revision 1
# baseline (speedup 1.0000x reference)
"""AdaEquiLayerNorm on Trainium2 v2 — sorted-node one-hot-matmul modulation.

Host preprocessing (index/data movement only — all FLOPs stay on device):
  * nodes are sorted by graph id; input rows permuted on host, output rows
    un-permuted after download.  A 896-node super-tile then spans ~10
    consecutive graph ids, so the per-node mod-row gather collapses to a
    K=KWIN one-hot matmul against a KWIN-row contiguous slice of the mod
    table (delivered into PSUM in node layout -> no dma_gather descriptor
    generation on the critical path).
  * the one-hot masks are host-built index tensors (bf16), streamed with x.

Device (per core, 12544 nodes = 14 super-tiles of 896 = 128 part x 7):
  1. time-MLP mod table [1024, 256] bf16 in DRAM (cols s0 s1 s2 | shift128),
     exactly as the v1 kernel.
  2. ONE dma_gather of 14*KWIN rows pulls every super-tile's table window
     into SBUF (448 descriptors, ~4us, once).
  3. per super-tile: PE matmuls onehot^T @ window -> psum scales+shifts;
     stats: DVE bn_stats (l=0) + ACT big-op Squares + DVE tensor_reduce;
     apply: ACT per-j affine (l=0 scale/center), GPSIMD muls (l=1,2),
     shift add from PSUM.
Engine budget/st: DVE ~6.9us, ACT ~6.3us, GPSIMD ~5.5us, PE ~2.2us, under
the ~8.8us/st DMA pace -> DMA-bound.
"""

import sys
from contextlib import ExitStack

import numpy as np
import ml_dtypes

try:
    import concourse.bass as bass
except ImportError:  # pragma: no cover
    sys.path.insert(0, "/opt/trn_rl_repo")
    import concourse.bass as bass

import concourse.mybir as mybir
import concourse.tile as tile
from concourse.bacc import Bacc
from concourse.bass_utils import run_bass_kernel_spmd

F32 = mybir.dt.float32
BF16 = mybir.dt.bfloat16
I16 = mybir.dt.int16
AF = mybir.ActivationFunctionType
ALU = mybir.AluOpType

N_FULL = 100000
D_IN = 480            # 128 (l=0) + 192 (64x l=1) + 160 (32x l=2)
B = 1024
TIME = 512
N_CORES = 8
PER_CORE = 12544      # 98 tiles of 128 nodes
T_TILES = 7           # node rows per partition per super-tile (896 nodes/st)
EPS = 1e-5
MAGIC = 12582912.0    # 1.5 * 2^23 — fp32 add/sub rounds to nearest integer
TWO_PI = float(2.0 * np.pi)
TBL_W = 256           # bf16 table row: [s0 s1 s2 | shift(128) | zeros(125)]
BB = 256              # per-core graph band: covers every window of the core
KWIN = 64             # graph-window rows per super-tile (one-hot K);
                      # 64 keeps every window at PE base partition 0 or 64


def _bcast(ap_slice: bass.AP, count: int) -> bass.AP:
    """[.., 1] slice -> [.., count] via a stride-0 innermost dim."""
    a = [list(x) for x in ap_slice.ap]
    assert a[-1][1] == 1, a
    a[-1] = [0, count]
    return bass.AP(tensor=ap_slice.tensor, offset=ap_slice.offset, ap=a)


def build_nc(
    n_nodes: int = PER_CORE, t_tiles: int = T_TILES, kwin: int = KWIN,
    native_silu: bool = True, psum_adds_on_gpsimd: bool = False,
    mod_interleave: bool = False,
) -> bass.Bass:
    stn = t_tiles * 128           # nodes per super-tile
    assert n_nodes % stn == 0
    n_st = n_nodes // stn
    n_win = n_st * kwin           # gathered table rows total
    assert n_win % 16 == 0

    nc = Bacc(dynamic_dma_scratch_size=14352)
    x_ext = nc.declare_dram_parameter("node_input", [n_nodes, D_IN], F32, isOutput=False)
    oh_ext = nc.declare_dram_parameter("onehot", [n_st, kwin, stn], BF16, isOutput=False)
    widx_ext = nc.declare_dram_parameter("widx", [128, max(n_win // 16, 1)], I16, isOutput=False)
    t_ext = nc.declare_dram_parameter("t", [BB], F32, isOutput=False)
    w1_ext = nc.declare_dram_parameter("w1", [256, TIME], BF16, isOutput=False)
    b1_ext = nc.declare_dram_parameter("b1", [TIME], F32, isOutput=False)
    w2_ext = nc.declare_dram_parameter("w2", [TIME, TIME], BF16, isOutput=False)
    b2_ext = nc.declare_dram_parameter("b2", [TIME], F32, isOutput=False)
    wmp_ext = nc.declare_dram_parameter("wmp", [TIME, TBL_W], BF16, isOutput=False)
    bmp_ext = nc.declare_dram_parameter("bmp", [TBL_W], F32, isOutput=False)
    out_ext = nc.declare_dram_parameter("out", [n_nodes, D_IN], F32, isOutput=True)

    table = nc.dram_tensor("mod_table", [BB, TBL_W], BF16)

    freqs = np.exp(-np.log(10000.0) * np.arange(128, dtype=np.float64) / 128.0)
    f2pi_const = nc.inline_tensor(
        (freqs / (2.0 * np.pi)).astype(np.float32).reshape(128, 1), name="f2pi"
    )

    def bcast_part(handle_ap: bass.AP, parts: int = 128) -> bass.AP:
        return bass.AP(
            tensor=handle_ap.tensor,
            offset=handle_ap.offset,
            ap=[[0, parts]] + list(handle_ap.ap),
        )

    with tile.TileContext(nc) as tc, ExitStack() as ctx:
        const = ctx.enter_context(tc.tile_pool(name="const", bufs=1))
        xio = ctx.enter_context(tc.tile_pool(name="xio", bufs=9))
        ohio = ctx.enter_context(tc.tile_pool(name="ohio", bufs=3))
        sm = ctx.enter_context(tc.tile_pool(name="sm", bufs=4))

        # ---- constants / weights into SBUF (weights cast to bf16 via SWDGE) ----
        f2pi_sb = const.tile([128, 1], F32)
        nc.gpsimd.dma_start(out=f2pi_sb, in_=f2pi_const[:, :])
        t_bc = const.tile([128, BB], F32)
        nc.gpsimd.dma_start(out=t_bc, in_=bcast_part(t_ext[:]))
        widx_sb = const.tile([128, max(n_win // 16, 1)], I16)
        nc.sync.dma_start(out=widx_sb, in_=widx_ext[:, :])
        w1_sb = const.tile([128, 2, TIME], BF16)
        nc.sync.dma_start(out=w1_sb, in_=w1_ext[:, :].rearrange("(k p) d -> p k d", p=128))
        w2_sb = const.tile([128, 4, TIME], BF16)
        nc.sync.dma_start(out=w2_sb, in_=w2_ext[:, :].rearrange("(k p) d -> p k d", p=128))
        wmp_sb = const.tile([128, 4, TBL_W], BF16)
        nc.sync.dma_start(out=wmp_sb, in_=wmp_ext[:, :].rearrange("(k p) d -> p k d", p=128))
        b1_sb = const.tile([128, 4], F32)
        nc.sync.dma_start(out=b1_sb, in_=b1_ext[:].rearrange("(m p) -> p m", p=128))
        b2_sb = const.tile([128, 4], F32)
        nc.sync.dma_start(out=b2_sb, in_=b2_ext[:].rearrange("(m p) -> p m", p=128))
        bmp_row = const.tile([1, TBL_W], F32)
        nc.sync.dma_start(out=bmp_row, in_=bmp_ext[None, :])
        ones_sb = const.tile([1, 128], BF16)
        nc.vector.memset(ones_sb, 1.0)
        eps_sb = const.tile([128, 1], F32)
        nc.vector.memset(eps_sb, EPS)
        quarter_sb = const.tile([128, 1], F32)
        nc.vector.memset(quarter_sb, 0.25)
        magic_sb = const.tile([128, 1], F32)
        nc.vector.memset(magic_sb, MAGIC)
        nmagic_sb = const.tile([128, 1], F32)
        nc.vector.memset(nmagic_sb, -MAGIC)

        # prefetch the first node super-tiles while the table is being built
        def x_view(st):
            rows = slice(st * stn, (st + 1) * stn)
            return x_ext[rows, :].rearrange("(p t) c -> p t c", t=t_tiles)

        oh_tiles = {}

        def emit_oh(st):
            p0 = (st * kwin) % 128
            oh = ohio.tile([128, stn], BF16, tag="oh", name=f"oh{st}")
            nc.sync.dma_start(out=oh[p0:p0 + kwin, :], in_=oh_ext[st, :, :])
            oh_tiles[st] = oh

        x_tiles = {}
        for st in range(min(9, n_st)):
            x_tiles[st] = xio.tile([128, t_tiles, D_IN], F32, tag="x", name=f"x{st}")
            nc.sync.dma_start(out=x_tiles[st], in_=x_view(st))

        # ---- table stage (scoped pool; SBUF released before the main loop) ----
        with tc.tile_pool(name="tbl", bufs=1) as tbl, \
                tc.tile_pool(name="tpsum", bufs=2, space="PSUM") as psum, \
                ExitStack() as ctx2:
            # embT[h][j, b] = cos/sin(t[b]*freqs[j]), bf16, via range-reduced
            # Sin.  The multiply/round chain runs on ACT (idle during the
            # head) with per-partition scale=f2pi; only the subtract is DVE.
            embs = ctx2.enter_context(tc.tile_pool(name="embs", bufs=1))
            m2 = embs.tile([128, 2, BB], F32, tag="m2")
            zero_sb = const.tile([128, 1], F32)
            nc.vector.memset(zero_sb, 0.0)
            nc.scalar.activation(out=m2[:, 1, :], in_=t_bc, func=AF.Identity,
                                 scale=f2pi_sb, bias=zero_sb)
            nc.scalar.activation(out=m2[:, 0, :], in_=m2[:, 1, :],
                                 func=AF.Identity, bias=quarter_sb)
            r2 = embs.tile([128, 2, BB], F32, tag="r2")
            nc.vector.tensor_scalar_add(out=r2, in0=m2, scalar1=MAGIC)
            nc.vector.tensor_scalar_sub(out=r2, in0=r2, scalar1=MAGIC)
            nc.vector.tensor_sub(out=m2, in0=m2, in1=r2)
            emb2 = tbl.tile([128, 2, BB], BF16, tag="emb2")
            nc.scalar.activation(out=emb2, in_=m2, func=AF.Sin, scale=TWO_PI)
            embT = [emb2[:, 0, :], emb2[:, 1, :]]
            ctx2.close()  # release m2/r2 before the MLP tiles allocate

            def silu_from_psum(out_ap, psum_ap, bias_ap):
                if native_silu:
                    nc.scalar.activation(
                        out=out_ap, in_=psum_ap, func=AF.Silu, bias=bias_ap, scale=1.0
                    )
                else:  # CoreSim fallback: silu(x) = x * sigmoid(x)
                    w = psum_ap.free_size()
                    lin = sm.tile([128, w], F32, tag="silu_lin")
                    nc.scalar.activation(
                        out=lin, in_=psum_ap, func=AF.Identity, bias=bias_ap, scale=1.0
                    )
                    sig = sm.tile([128, w], F32, tag="silu_sig")
                    nc.scalar.activation(out=sig, in_=lin, func=AF.Sigmoid)
                    nc.vector.tensor_mul(out=out_ap, in0=lin, in1=sig)

            # s1 = silu(emb @ w1 + b1)^T   [512(4 ptiles), BB], bf16
            s1 = tbl.tile([128, 4, BB], BF16)
            for mi in range(4):
                ps = psum.tile([128, BB], F32, tag="mlp", bufs=4)
                for k in range(2):
                    nc.tensor.matmul(
                        ps, w1_sb[:, k, mi * 128:(mi + 1) * 128],
                        embT[k][:, :],
                        start=(k == 0), stop=(k == 1),
                    )
                silu_from_psum(s1[:, mi, :], ps, b1_sb[:, mi:mi + 1])
            # s2 = silu(s1^T @ w2 + b2)^T  (= silu(t_emb), fused), bf16
            # mod k-chunk matmuls are interleaved right after each s2
            # row-block mi completes, hiding the mod stage behind s2.
            MODW = 132
            s2 = tbl.tile([128, 4, BB], BF16)
            bmp_bf = tbl.tile([1, MODW], BF16)
            nc.vector.tensor_copy(out=bmp_bf, in_=bmp_row[:, 0:MODW])
            for mi in range(4):
                ps = psum.tile([128, BB], F32, tag="mlp", bufs=4)
                for k in range(4):
                    nc.tensor.matmul(
                        ps, w2_sb[:, k, mi * 128:(mi + 1) * 128],
                        s1[:, k, :],
                        start=(k == 0), stop=(k == 3),
                    )
                silu_from_psum(s2[:, mi, :], ps, b2_sb[:, mi:mi + 1])
            msb = tbl.tile([128, BB // 128, TBL_W], BF16)
            nc.vector.memset(msb, 0.0)
            for bc in range(BB // 128):
                psm = psum.tile([128, MODW], F32, tag="mod", bufs=2)
                for mi in range(4):
                    nc.tensor.matmul(
                        psm, s2[:, mi, bc * 128:(bc + 1) * 128],
                        wmp_sb[:, mi, 0:MODW], start=(mi == 0), stop=False,
                    )
                nc.tensor.matmul(psm, ones_sb, bmp_bf, start=False, stop=True)
                nc.vector.tensor_copy(out=msb[:, bc, 0:MODW], in_=psm)
            nc.scalar.dma_start(
                out=table[:, :].rearrange("(c p) w -> p c w", p=128), in_=msb)
            # table rows: mod[b, :] = silu(t_emb)[b] @ wmp + bmp  (bf16 in DRAM)
        mpsum = ctx.enter_context(tc.tile_pool(name="mpsum", bufs=4, space="PSUM"))

        # ---- one gather pulls every super-tile's kwin-row table window ----
        # row st*kwin + r lands at partition (st*kwin + r) % 128, free slot
        # (st*kwin + r) // 128 -> window of st = partitions [st*kwin % 128,
        # +kwin) at free slot st*kwin // 128.
        tsl = const.tile([128, (n_win + 127) // 128, TBL_W], BF16)
        nc.gpsimd.dma_gather(
            out_ap=tsl[:],
            in_ap=table[:, :],
            idxs_ap=widx_sb[:, :],
            num_idxs=n_win,
            num_idxs_reg=n_win,
            elem_size=TBL_W,
            single_packet=True,
        )

        def win_rhs(st, c0, c1):
            p0 = (st * kwin) % 128
            f0 = (st * kwin) // 128
            return tsl[p0:p0 + kwin, f0, c0:c1]

        # ---- main loop, software-pipelined by one super-tile ----
        sc1 = 1.0 / np.sqrt(192.0)  # Square(x*sc) accumulates ssq/192 directly
        sc2 = 1.0 / np.sqrt(160.0)
        state = {}

        def emit_stats(st):
            if st not in x_tiles:
                x_tiles[st] = xio.tile([128, t_tiles, D_IN], F32, tag="x", name=f"x{st}")
                nc.sync.dma_start(out=x_tiles[st], in_=x_view(st))
            x_sb = x_tiles[st]
            st6 = sm.tile([128, t_tiles, 6], F32, tag="st6")
            for ti in range(t_tiles):
                nc.vector.bn_stats(out=st6[:, ti, :], in_=x_sb[:, ti, 0:128])
            vvv = sm.tile([128, t_tiles, 4], F32, tag="vvv", bufs=8)
            for ti in range(t_tiles):
                nc.vector.bn_aggr(out=vvv[:, ti, 0:2], in_=st6[:, ti, :])
            # big-op squares (bf16 scratch) + per-ti reduces
            sq1 = sm.tile([128, t_tiles, 192], BF16, tag="sq1", bufs=2)
            nc.scalar.activation(out=sq1, in_=x_sb[:, :, 128:320], func=AF.Square,
                                 scale=sc1)
            sq2 = sm.tile([128, t_tiles, 160], BF16, tag="sq2", bufs=2)
            nc.scalar.activation(out=sq2, in_=x_sb[:, :, 320:480], func=AF.Square,
                                 scale=sc2)
            nc.vector.tensor_reduce(out=vvv[:, :, 2:3], in_=sq1,
                                    axis=mybir.AxisListType.X, op=ALU.add)
            nc.vector.tensor_reduce(out=vvv[:, :, 3:4], in_=sq2,
                                    axis=mybir.AxisListType.X, op=ALU.add)
            ivv = sm.tile([128, t_tiles, 3], F32, tag="ivv", bufs=8)
            nc.vector.tensor_scalar_add(out=ivv, in0=vvv[:, :, 1:4], scalar1=EPS)
            nc.vector.reciprocal(out=ivv, in_=ivv)
            rr = sm.tile([128, t_tiles, 3], F32, tag="rr", bufs=8)
            nc.scalar.activation(out=rr, in_=ivv, func=AF.Sqrt)  # rsqrt(v+eps)
            state[st] = (x_sb, vvv, rr)

        def emit_modmm(st):
            p0 = (st * kwin) % 128
            if st not in oh_tiles:
                emit_oh(st)
            oh = oh_tiles.pop(st)
            # one 2-bank tile: slots 0..6 = per-jb shifts, slot 7 = 7x[128,4]
            # scale strips (every matmul dst region stays inside one bank)
            mp = mpsum.tile([128, 8, 128], F32, tag="mp")
            for jb in range(t_tiles):
                lhsT = oh[p0:p0 + kwin, jb * 128:(jb + 1) * 128]
                nc.tensor.matmul(mp[:, jb, :], lhsT, win_rhs(st, 3, 131),
                                 start=True, stop=True)
                nc.tensor.matmul(mp[:, 7, 4 * jb:4 * jb + 4], lhsT,
                                 win_rhs(st, 0, 4), start=True, stop=True)
            return mp

        def emit_amul(st, mp):
            _, vvv, rr = state[st]
            scl = mp[:, 7, 0:25]
            psS3 = bass.AP(tensor=scl.tensor, offset=scl.offset,
                           ap=[list(scl.ap[0]), [4, t_tiles], [1, 3]])
            # amul = (1 + s) * rr ; nbmn = -mean * amul0
            amul = sm.tile([128, t_tiles, 3], F32, tag="amul", bufs=8)
            nc.vector.scalar_tensor_tensor(
                out=amul, in0=psS3, scalar=1.0, in1=rr,
                op0=ALU.add, op1=ALU.mult)
            nbmn = sm.tile([128, t_tiles, 1], F32, tag="nbmn", bufs=8)
            nc.vector.scalar_tensor_tensor(
                out=nbmn, in0=vvv[:, :, 0:1], scalar=-1.0, in1=amul[:, :, 0:1],
                op0=ALU.mult, op1=ALU.mult)
            return (amul, nbmn)

        def emit_apply(st, mods):
            x_sb, vvv, rr = state.pop(st)
            mp, (amul, nbmn) = mods
            # irrep0 affine on ACT: x0 = x0*amul0 - mean*amul0, per-j scale/bias
            for jb in range(t_tiles):
                nc.scalar.activation(
                    out=x_sb[:, jb, 0:128], in_=x_sb[:, jb, 0:128], func=AF.Identity,
                    scale=amul[:, jb, 0:1], bias=nbmn[:, jb, :])
            # irrep1/2 scaling on GPSIMD (in-place)
            nc.gpsimd.tensor_tensor(
                out=x_sb[:, :, 128:320], in0=x_sb[:, :, 128:320],
                in1=_bcast(amul[:, :, 1:2], 192), op=ALU.mult,
            )
            nc.gpsimd.tensor_tensor(
                out=x_sb[:, :, 320:480], in0=x_sb[:, :, 320:480],
                in1=_bcast(amul[:, :, 2:3], 160), op=ALU.mult,
            )
            # shift add from PSUM (DVE: GPSIMD cannot read PSUM)
            nc.vector.tensor_tensor(out=x_sb[:, :, 0:128], in0=x_sb[:, :, 0:128],
                                    in1=mp[:, 0:t_tiles, :], op=ALU.add)
            rows = slice(st * stn, (st + 1) * stn)
            nc.sync.dma_start(
                out=out_ext[rows, :].rearrange("(p t) c -> p t c", t=t_tiles),
                in_=x_sb,
            )

        # Front-load stats for every buffered super-tile so no engine queue
        # head-blocks on the (late) mod table; then interleave apply(st) /
        # stats(st+8) / modmm(st+2).
        n_pre = min(9, n_st)
        for st in range(n_pre):
            emit_stats(st)
        mods = {}
        for st in range(min(4, n_st)):
            mp = emit_modmm(st)
            mods[st] = (mp, emit_amul(st, mp))
        for st in range(n_st):
            emit_apply(st, mods.pop(st))
            if st - 1 + n_pre < n_st and st >= 1:
                emit_stats(st - 1 + n_pre)
            if st + 4 < n_st:
                mp = emit_modmm(st + 4)
                mods[st + 4] = (mp, emit_amul(st + 4, mp))
        if n_st > n_pre:
            pass

    nc.finalize()
    return nc


def _prep_in_maps(node_input, t, batch, w1, b1, w2, b2, wm, bm, n_nodes=PER_CORE,
                  t_tiles=T_TILES, kwin=KWIN):
    """Sort nodes by graph, shard, build one-hot + window-idx tensors.

    Returns (in_maps, starts, sort_idx) where starts are offsets into the
    SORTED array and sort_idx maps sorted row -> original row.
    """
    stn = t_tiles * 128
    n_st = n_nodes // stn
    n_win = n_st * kwin
    wmp = np.zeros((TIME, TBL_W), np.float32)
    wmp[:, 0:3] = wm[:, 0:3]
    wmp[:, 3:131] = wm[:, 224:352]
    bmp = np.zeros((TBL_W,), np.float32)
    bmp[0:3] = bm[0:3]
    bmp[3:131] = bm[224:352]
    shared = {
        "t": np.ascontiguousarray(t, dtype=np.float32),
        "w1": np.ascontiguousarray(w1).astype(ml_dtypes.bfloat16),
        "b1": np.ascontiguousarray(b1, dtype=np.float32),
        "w2": np.ascontiguousarray(w2).astype(ml_dtypes.bfloat16),
        "b2": np.ascontiguousarray(b2, dtype=np.float32),
        "wmp": wmp.astype(ml_dtypes.bfloat16),
        "bmp": bmp,
    }
    n = node_input.shape[0]
    del shared["t"]
    sort_idx = np.argsort(batch, kind="stable")
    x_sorted = np.ascontiguousarray(node_input[sort_idx], dtype=np.float32)
    b_sorted = np.asarray(batch)[sort_idx].astype(np.int32)

    starts = [min(i * n_nodes, n - n_nodes) for i in range(N_CORES)]
    in_maps = []
    for s in starts:
        bs = b_sorted[s:s + n_nodes]
        g0c = min(int(bs[0]), B - BB)
        assert int(bs[-1]) - g0c < BB, "graph band exceeds BB"
        t_band = np.ascontiguousarray(
            np.asarray(t, dtype=np.float32)[g0c:g0c + BB])
        onehot = np.zeros((n_st, kwin, stn), dtype=np.float32)
        glo = np.zeros((n_st,), np.int32)
        for st in range(n_st):
            seg = bs[st * stn:(st + 1) * stn]
            g0 = min(int(seg[0]), B - kwin)
            rng = int(seg[-1]) - g0 + 1
            assert rng <= kwin, f"graph window {rng} exceeds kwin={kwin}"
            glo[st] = g0
            # one-hot column j = jb*128 + p corresponds to node p*t_tiles + jb
            k_of_node = (seg - g0).reshape(128, t_tiles)      # [p, jb]
            cols = k_of_node.T.reshape(-1)                    # j = jb*128+p
            onehot[st, cols, np.arange(stn)] = 1.0
        # window gather idx (band-relative): i = st*kwin + r -> row
        # glo[st]-g0c+r, wrapped idx[i % 16, i // 16], replicated x8 groups
        wi = (glo[:, None] - g0c + np.arange(kwin)[None, :]).reshape(-1).astype(np.int16)
        idx16 = wi.reshape(n_win // 16, 16).T                 # [16, n_win/16]
        widx = np.ascontiguousarray(np.tile(idx16, (8, 1)))
        in_maps.append(
            {
                **shared,
                "t": t_band,
                "node_input": np.ascontiguousarray(x_sorted[s:s + n_nodes]),
                "onehot": onehot.astype(ml_dtypes.bfloat16),
                "widx": widx,
            }
        )
    return in_maps, starts, sort_idx


_NC_CACHE: dict = {}


def _get_nc(n_nodes=PER_CORE, t_tiles=T_TILES, kwin=KWIN):
    key = (n_nodes, t_tiles, kwin)
    if key not in _NC_CACHE:
        _NC_CACHE[key] = build_nc(n_nodes, t_tiles, kwin)
    return _NC_CACHE[key]


def run(node_input, t, batch, w1, b1, w2, b2, wm, bm, trace=False, **trace_kwargs):
    """Run on 8 NeuronCores; returns (full output, BassKernelResults)."""
    node_input = np.asarray(node_input)
    n = node_input.shape[0]
    in_maps, starts, sort_idx = _prep_in_maps(
        node_input, np.asarray(t), np.asarray(batch),
        np.asarray(w1), np.asarray(b1), np.asarray(w2), np.asarray(b2),
        np.asarray(wm), np.asarray(bm),
    )
    nc = _get_nc()
    res = run_bass_kernel_spmd(
        nc, in_maps, core_ids=list(range(N_CORES)), trace=trace, **trace_kwargs
    )
    out_sorted = np.empty((n, D_IN), dtype=np.float32)
    for s, core_res in zip(starts, res.results):
        out_sorted[s:s + PER_CORE] = core_res["out"]
    out = np.empty((n, D_IN), dtype=np.float32)
    out[sort_idx] = out_sorted
    return out, res


def kernel(node_input, t, batch, w1, b1, w2, b2, wm, bm):
    out, _ = run(node_input, t, batch, w1, b1, w2, b2, wm, bm, trace=False)
    return out

